# revision 12
# baseline (speedup 1.0000x reference)
"""Trainium2 Bass kernel for CharRep: ragged bidirectional char-LSTM encoder.

Reference computes, for N = 64*256 = 16384 sequences of T=16 char tokens:
  x = emb[ids]; fwd LSTM -> hidden at position (nonzero_count-1 clamped >= 0);
  rev LSTM over reversed tokens -> final hidden; concat -> [64, 256, 100].

Device strategy (data-parallel over 8 cores, 2048 rows each):
  - Fold emb @ w_ih.T + biases into a [vocab=100, 4H] gate table per
    direction (host precompute; weight-only transform).
  - Token gate-preactivations injected with one-hot matmuls. The one-hot is
    host-encoded and K-stacked bidirectionally: for scan step s, rows hold
    onehot(ids[s]) for the fwd direction and onehot(ids[15-s]) for the rev
    direction, split across two tensors (vocab 0:64 -> ohA with K=128,
    vocab 64:100 -> ohB with K=72) to fit the PE's K<=128.
  - psum[gate] = tabA.T @ ohA_s + tabB.T @ ohB_s + rec.T @ h_{t-1}  (PSUM
    accumulation, all full-partition groups).
  - Layout: rows on the free axis, gates/hidden on partitions; the two
    directions are packed into one 128-partition tile (fwd lanes 0:50,
    rev lanes 64:114) so ACT/DVE ops cover both directions per pass.
  - Forward ragged selection: mask (idx == t) + copy_predicated each step.
  - Output: PE transpose [50, 128] -> [128, 50] chunks, concat, DMA out.
"""

import sys

sys.path.insert(0, "/opt/trn_rl_repo")

import numpy as np
import ml_dtypes

bf16 = ml_dtypes.bfloat16

N_CORES = 8
B, S, T = 64, 256, 16
NTOT = B * S
N = NTOT // N_CORES  # 2048 rows per core
H = 50
V = 100  # vocab
VA = 64  # vocab rows in the A-stack (per direction)
VB = V - VA  # 36; B-stack K = 72
CH = 512  # matmul row-chunk (free dim)
NCH = N // CH
G = 64  # padded per-direction gate lane stride

_CACHE = {}


def _build_nc():
    import concourse.bass as bass
    import concourse.bacc as bacc
    import concourse.mybir as mybir
    from concourse import tile
    from contextlib import ExitStack

    dt = mybir.dt
    BF = dt.bfloat16
    F32 = dt.float32
    AF = mybir.ActivationFunctionType
    ALU = mybir.AluOpType

    nc = bacc.Bacc("TRN2", target_bir_lowering=False, debug=False, num_devices=N_CORES)

    d_ids32 = nc.declare_dram_parameter("ids32", [T, N], dt.int32, isOutput=False)
    d_ohA = nc.declare_dram_parameter("ohA", [2 * VA, T, N], BF, isOutput=False)
    d_ohB = nc.declare_dram_parameter("ohB", [2 * VB, T, N], BF, isOutput=False)
    d_tabA = nc.declare_dram_parameter("tabA", [2 * VA, 4 * 128], BF, isOutput=False)
    d_tabB = nc.declare_dram_parameter("tabB", [2 * VB, 4 * 128], BF, isOutput=False)
    d_rec = nc.declare_dram_parameter("rec", [128, 4 * 128], BF, isOutput=False)
    d_ones = nc.declare_dram_parameter("ones16", [T, G], F32, isOutput=False)
    d_ident = nc.declare_dram_parameter("ident", [64, 64], BF, isOutput=False)
    d_out = nc.declare_dram_parameter("out", [N, 2 * H], F32, isOutput=True)

    with ExitStack() as ctx:
        tc = ctx.enter_context(tile.TileContext(nc))
        const = ctx.enter_context(tc.tile_pool(name="const", bufs=1))
        prep = ctx.enter_context(tc.tile_pool(name="prep", bufs=2))
        state = ctx.enter_context(tc.tile_pool(name="state", bufs=2))
        work = ctx.enter_context(tc.tile_pool(name="work", bufs=4))
        gf16 = ctx.enter_context(tc.tile_pool(name="gf16", bufs=4))
        outp = ctx.enter_context(tc.tile_pool(name="outp", bufs=3))
        pp_big = ctx.enter_context(tc.tile_pool(name="pp_big", bufs=2, space="PSUM"))
        pp_sm = ctx.enter_context(tc.tile_pool(name="pp_sm", bufs=2, space="PSUM"))

        # ---- constants / inputs to SBUF ----
        t_ids32 = const.tile([T, N], dt.int32)
        t_tabA = const.tile([2 * VA, 4 * 128], BF)
        t_tabB = const.tile([2 * VB, 4 * 128], BF)
        t_rec = const.tile([128, 4 * 128], BF)
        t_ones = const.tile([T, G], F32)
        t_ident = const.tile([64, 64], BF)
        for tt, dd in [
            (t_ids32, d_ids32),
            (t_tabA, d_tabA),
            (t_tabB, d_tabB),
            (t_rec, d_rec),
            (t_ones, d_ones),
            (t_ident, d_ident),
        ]:
            nc.sync.dma_start(tt[:], dd[:])

        # one-hot slices are streamed from HBM just-in-time (see scan loop)
        ohp = ctx.enter_context(tc.tile_pool(name="ohp", bufs=10))

        # ---- ragged lengths -> idx = max(len-1, 0), on 64 lanes ----
        t_nz = prep.tile([T, N], F32)
        nc.vector.tensor_scalar(t_nz[:], t_ids32[:], 0, None, ALU.not_equal)
        t_idxrep = const.tile([G, N], BF)
        for c in range(NCH):
            cs = slice(c * CH, (c + 1) * CH)
            ps_len = pp_sm.tile([G, CH], F32, tag="pp_sm")
            nc.tensor.matmul(ps_len[:], t_ones[:], t_nz[:, cs], start=True, stop=True)
            nc.vector.tensor_scalar(
                t_idxrep[:, cs], ps_len[:], 1.0, 0.0, ALU.subtract, ALU.max
            )

        # ---- initial state ----
        t_hsel = const.tile([G, N], BF)
        nc.vector.memset(t_hsel[:], 0.0)
        h_prev = state.tile([128, N], BF, tag="h")
        c_prev = state.tile([128, N], BF, tag="c")
        nc.vector.memset(h_prev[:], 0.0)
        nc.vector.memset(c_prev[:], 0.0)

        # gate order in weights: i, f, g, o.  psum_ifo slots: [i, f, o]; g alone.
        IFO = (0, 1, 3)

        def gate_mms(tgt, X, ohA, ohB, h_rhs):
            nc.tensor.matmul(
                tgt,
                t_tabA[:, X * 128 : (X + 1) * 128],
                ohA[:],
                start=True,
                stop=False,
            )
            nc.tensor.matmul(
                tgt,
                t_tabB[:, X * 128 : (X + 1) * 128],
                ohB[:],
                start=False,
                stop=False,
            )
            nc.tensor.matmul(
                tgt,
                t_rec[:, X * 128 : (X + 1) * 128],
                h_rhs,
                start=False,
                stop=True,
            )

        # ---- the scan ----
        for s in range(T):
            mask = work.tile([G, N], dt.uint8, tag="mask")
            nc.vector.tensor_scalar(
                mask[:], t_idxrep[:], float(s), None, ALU.is_equal
            )
            h_new = state.tile([128, N], BF, tag="h")
            c_new = state.tile([128, N], BF, tag="c")
            for c in range(NCH):
                cs = slice(c * CH, (c + 1) * CH)
                ohA = ohp.tile([2 * VA, CH], BF, tag="ohA")
                nc.sync.dma_start(ohA[:], d_ohA[:, s, cs])
                ohB = ohp.tile([2 * VB, CH], BF, tag="ohB")
                nc.sync.dma_start(ohB[:], d_ohB[:, s, cs])
                ps_ifo = pp_big.tile([128, 3 * CH], F32, tag="pp_big")
                ps_g = pp_sm.tile([128, CH], F32, tag="pp_sm")
                for slot, X in enumerate(IFO):
                    gate_mms(
                        ps_ifo[:, slot * CH : (slot + 1) * CH],
                        X, ohA, ohB, h_prev[:, cs],
                    )
                gate_mms(ps_g[:], 2, ohA, ohB, h_prev[:, cs])

                gifo = gf16.tile([128, 3 * CH], BF, tag="gifo")
                nc.scalar.activation(gifo[:], ps_ifo[:], AF.Sigmoid)
                gg = gf16.tile([128, CH], BF, tag="gg")
                nc.scalar.activation(gg[:], ps_g[:], AF.Tanh)

                t1 = work.tile([128, CH], BF, tag="t1")
                nc.vector.tensor_tensor(t1[:], gifo[:, 0:CH], gg[:], ALU.mult)
                t2 = work.tile([128, CH], BF, tag="t2")
                nc.vector.tensor_tensor(
                    t2[:], gifo[:, CH : 2 * CH], c_prev[:, cs], ALU.mult
                )
                nc.vector.tensor_tensor(c_new[:, cs], t1[:], t2[:], ALU.add)
                tch = work.tile([128, CH], BF, tag="tch")
                nc.scalar.activation(tch[:], c_new[:, cs], AF.Tanh)
                nc.vector.tensor_tensor(
                    h_new[:, cs], gifo[:, 2 * CH : 3 * CH], tch[:], ALU.mult
                )
                nc.vector.copy_predicated(
                    t_hsel[:, cs], mask[:, cs], h_new[0:G, cs]
                )
            h_prev, c_prev = h_new, c_new

        # ---- output: transpose [50, 128] chunks -> [128, 50], concat, DMA ----
        h_rev = const.tile([G, N], BF)
        nc.vector.tensor_copy(h_rev[:], h_prev[G:128, :])
        for c in range(N // 128):
            cs = slice(c * 128, (c + 1) * 128)
            ps_tf = pp_sm.tile([128, H], BF, tag="pp_sm")
            nc.tensor.transpose(ps_tf[:], t_hsel[0:H, cs], t_ident[0:H, 0:H])
            ps_tr = pp_sm.tile([128, H], BF, tag="pp_sm")
            nc.tensor.transpose(ps_tr[:], h_rev[0:H, cs], t_ident[0:H, 0:H])
            stage = outp.tile([128, 2 * H], F32, tag="stage")
            nc.vector.tensor_copy(stage[:, 0:H], ps_tf[:])
            nc.vector.tensor_copy(stage[:, H : 2 * H], ps_tr[:])
            nc.sync.dma_start(d_out[cs, :], stage[:])

    nc.finalize()
    return nc


def _host_prep(inputs):
    """Build per-core input maps from the full problem inputs."""
    ids = np.asarray(inputs["char_ids"]).reshape(NTOT, T).astype(np.int64)
    emb = np.asarray(inputs["emb"], dtype=np.float32)

    def table(w_ih, b_ih, b_hh):
        return emb @ np.asarray(w_ih, np.float32).T + np.asarray(
            b_ih, np.float32
        ) + np.asarray(b_hh, np.float32)

    tb_f = table(inputs["w_ih_f"], inputs["b_ih_f"], inputs["b_hh_f"])
    tb_r = table(inputs["w_ih_r"], inputs["b_ih_r"], inputs["b_hh_r"])

    # stacked stationaries: tabA [2*VA, 4*128], tabB [2*VB, 4*128]
    tabA = np.zeros((2 * VA, 4 * 128), np.float32)
    tabB = np.zeros((2 * VB, 4 * 128), np.float32)
    for X in range(4):
        gf = tb_f[:, X * H : (X + 1) * H]  # [V, 50]
        gr = tb_r[:, X * H : (X + 1) * H]
        tabA[0:VA, X * 128 + 0 : X * 128 + H] = gf[0:VA]
        tabA[VA : 2 * VA, X * 128 + G : X * 128 + G + H] = gr[0:VA]
        tabB[0:VB, X * 128 + 0 : X * 128 + H] = gf[VA:V]
        tabB[VB : 2 * VB, X * 128 + G : X * 128 + G + H] = gr[VA:V]
    tabA = tabA.astype(bf16)
    tabB = tabB.astype(bf16)

    rec = np.zeros((128, 4 * 128), np.float32)
    whf = np.asarray(inputs["w_hh_f"], np.float32)
    whr = np.asarray(inputs["w_hh_r"], np.float32)
    for X in range(4):
        rec[0:H, X * 128 : X * 128 + H] = whf[X * H : (X + 1) * H, :].T
        rec[G : G + H, X * 128 + G : X * 128 + G + H] = whr[X * H : (X + 1) * H, :].T
    rec = rec.astype(bf16)

    ones16 = np.ones((T, G), np.float32)
    ident = np.eye(64, dtype=np.float32).astype(bf16)

    vv = np.arange(V, dtype=np.int64)
    in_maps = []
    for i in range(N_CORES):
        rows = ids[i * N : (i + 1) * N]  # [N, T]
        tm = np.ascontiguousarray(rows.T)  # [T, N]
        tm_rev = tm[::-1]  # rev-direction token at scan step s is ids[T-1-s]
        ohA = np.empty((2 * VA, T, N), bf16)
        ohB = np.empty((2 * VB, T, N), bf16)
        ohA[0:VA] = (vv[0:VA, None, None] == tm[None, :, :]).astype(bf16)
        ohA[VA:] = (vv[0:VA, None, None] == tm_rev[None, :, :]).astype(bf16)
        ohB[0:VB] = (vv[VA:V, None, None] == tm[None, :, :]).astype(bf16)
        ohB[VB:] = (vv[VA:V, None, None] == tm_rev[None, :, :]).astype(bf16)
        in_maps.append(
            {
                "ids32": tm.astype(np.int32),
                "ohA": ohA,
                "ohB": ohB,
                "tabA": tabA,
                "tabB": tabB,
                "rec": rec,
                "ones16": ones16,
                "ident": ident,
            }
        )
    return in_maps


def kernel(**inputs):
    from concourse.bass_utils import run_bass_kernel_spmd

    if "nc" not in _CACHE:
        _CACHE["nc"] = _build_nc()
    nc = _CACHE["nc"]

    in_maps = _host_prep(inputs)
    res = run_bass_kernel_spmd(nc, in_maps, list(range(N_CORES)))
    outs = [np.asarray(res.results[i]["out"], np.float32) for i in range(N_CORES)]
    full = np.concatenate(outs, axis=0)  # [NTOT, 100]
    return full.reshape(B, S, 2 * H)
